# revision 2
# baseline (speedup 1.0000x reference)
"""Multi-head attention forward on 8 Trainium2 NeuronCores (Bass/Tile).

Problem: B=2, S=2048, d_model=1024, 16 heads (depth 64), fp32.
  q/k/v = query @ W{q,k,v}; logits = q k^T / 8 + mask * -1e9;
  out = softmax(logits) v @ Wo.

Sharding (Megatron-style, hardcoded): core c handles batch b = c//4 and head
group hg = c%4 (4 heads = 256 of the 1024 head dims). Wq/Wk/Wv are
column-sharded, Wo row-sharded; each core emits a partial [S, 1024] output and
the host sums the 4 partials per batch (the "all-reduce").

Per-core kernel design (all matmul operands bf16, f32 PSUM accumulate):
  * All attention math runs transposed: qT/kT are [depth, S] so QK^T lands as
    logitsT [k, q] tiles straight off the PE with no transposes, and
    AV^T = V.T(lhsT) @ expT needs none either. bf16 weights enable the PE's
    fast-weight-load path so LDWEIGHTS overlaps the previous matmul.
  * ScalarE turns logits psum directly into exp weights (scale folded in);
    the (1-mask) multiply runs on VectorE in bf16 (2x mode) with a slice of
    tiles offloaded to GpSimd to keep VectorE under the ScalarE exp wall.
  * The softmax denominator comes free from a ones-column appended to V
    (row 64 of the AV psum accumulator); V tiles are padded to 128 columns
    (zeros) so the AV weights qualify for fast weight load. The reciprocal
    runs directly on the psum row, GpSimd broadcasts it across 64 partitions,
    and the normalize is fused into the psum->sbuf eviction multiply.
  * The mask streams in 16 split DMAs issued during the projection phase so
    tiles land before the attention loop needs them; the output projection
    for each 1024-wide q-chunk is folded into the main loop so it overlaps
    the next chunk's attention; a short identity-matmul spin warms the PE
    clock (HAM) while the first DMAs land.
"""

import sys

import numpy as np

sys.path.insert(0, "/opt/trn_rl_repo")

B = 2
S = 2048
D = 1024
HEADS = 16
DEPTH = 64
CORES = 8
HG = 4          # head groups (cores per batch)
HPC = 4         # heads per core
DH = HPC * DEPTH  # per-core head width = 256

GPS_KB = (3, 8, 13)  # mask tiles handled by GpSimd instead of VectorE

_CACHE = {}


def _build_program():
    import concourse.bass as bass  # noqa: F401  (registers engines)
    import concourse.mybir as mybir
    import concourse.tile as tile
    from concourse import bacc, library_config
    from concourse.bass_interp import get_hw_module
    from concourse.masks import make_identity

    dt = mybir.dt
    f32, bf16 = dt.float32, dt.bfloat16
    MULT = mybir.AluOpType.mult
    EXP = mybir.ActivationFunctionType.Exp

    nc = bacc.Bacc(
        "TRN2",
        target_bir_lowering=False,
        debug=False,
        enable_asserts=True,
        num_devices=CORES,
    )

    xT = nc.dram_tensor("xT", [D, S], bf16, kind="ExternalInput").ap()
    imaskT = nc.dram_tensor("imaskT", [S, S], bf16, kind="ExternalInput").ap()
    wq = nc.dram_tensor("wq", [D, DH], bf16, kind="ExternalInput").ap()
    wk = nc.dram_tensor("wk", [D, DH], bf16, kind="ExternalInput").ap()
    wv = nc.dram_tensor("wv", [D, DH], bf16, kind="ExternalInput").ap()
    wo = nc.dram_tensor("wo", [DH, D], bf16, kind="ExternalInput").ap()
    vpad = nc.dram_tensor("vpad", [128, HPC, DEPTH], bf16, kind="ExternalInput").ap()
    out = nc.dram_tensor("out", [S, D], bf16, kind="ExternalOutput").ap()

    with tile.TileContext(nc) as tc:
        nc.gpsimd.load_library(library_config.proxy)
        with tc.tile_pool(name="persist", bufs=1) as pp:
            # Persistent SBUF tiles.
            qT = [pp.tile([128, S], bf16, tag=f"qT{g}", name=f"qT{g}") for g in range(2)]
            kT = [pp.tile([128, S], bf16, tag=f"kT{g}", name=f"kT{g}") for g in range(2)]
            # V tiles padded to 128 cols: [0:64]=V, [64]=ones, [65:128]=zeros.
            vt = [pp.tile([128, HPC, 128], bf16, tag=f"v{i}", name=f"v{i}") for i in range(16)]
            wot = [pp.tile([128, D], bf16, tag=f"wo{g}", name=f"wo{g}") for g in range(2)]
            mt = pp.tile([128, 16, S], bf16, tag="mask", name="mask")
            ident = pp.tile([128, 128], f32, tag="ident", name="ident")

            make_identity(nc, ident[:])
            with tc.tile_pool(name="psW", bufs=2, space="PSUM") as psW:
                for w in range(80):
                    psw = psW.tile([128, 128], f32, tag="warm", name="warm")
                    nc.tensor.matmul(psw[:], ident[:], ident[:],
                                     start=True, stop=True)

            # ---- Phase 1: projections (xT is query[b].T, fed transposed from host)
            with tc.tile_pool(name="xw", bufs=1) as xw, \
                 tc.tile_pool(name="psA", bufs=4, space="PSUM") as psA:
                xt = [xw.tile([128, S], bf16, tag=f"x{d}", name=f"x{d}") for d in range(8)]
                wts = {}
                for nm in ("wq", "wk", "wv"):
                    wts[nm] = [xw.tile([128, DH], bf16, tag=f"{nm}{d}", name=f"{nm}{d}") for d in range(8)]
                for d in range(8):
                    nc.sync.dma_start(wts["wq"][d][:], wq[d * 128:(d + 1) * 128, :])
                for d in range(8):
                    nc.sync.dma_start(xt[d][:], xT[d * 128:(d + 1) * 128, :])
                for nm, srcd in (("wk", wk), ("wv", wv)):
                    for d in range(8):
                        nc.sync.dma_start(wts[nm][d][:], srcd[d * 128:(d + 1) * 128, :])
                # Mask chunks stream while the PE runs projections; chunk kb is
                # first needed when the attention loop reaches (h0, kb).
                imaskT_r = imaskT.rearrange("(t p) q -> p t q", p=128)
                for kb in range(16):
                    nc.sync.dma_start(mt[:, kb:kb + 1, :], imaskT_r[:, kb:kb + 1, :])
                for g in range(2):
                    nc.sync.dma_start(wot[g][:], wo[g * 128:(g + 1) * 128, :])

                # qT/kT: [dh, s] = Wq^T-slice . xT, accumulated over 8 D-chunks.
                for wt, dst in ((wts["wq"], qT), (wts["wk"], kT)):
                    for g in range(2):
                        for sc in range(4):
                            ps = psA.tile([128, 512], f32, tag="proj", name="proj")
                            for d in range(8):
                                nc.tensor.matmul(
                                    ps[:],
                                    wt[d][:, g * 128:(g + 1) * 128],
                                    xt[d][:, sc * 512:(sc + 1) * 512],
                                    start=(d == 0), stop=(d == 7),
                                )
                            nc.vector.tensor_copy(dst[g][:, sc * 512:(sc + 1) * 512], ps[:])

                # v: natural [s, dh] layout, stored per 128-row tile as
                # [128, head, 128] with a ones column at index 64 and zero
                # padding above (denominator + fast-weight-load eligibility).
                for st in range(16):
                    ps = psA.tile([128, DH], f32, tag="proj", name="proj")
                    for d in range(8):
                        nc.tensor.matmul(
                            ps[:],
                            xt[d][:, st * 128:(st + 1) * 128],
                            wts["wv"][d][:],
                            start=(d == 0), stop=(d == 7),
                        )
                    nc.sync.dma_start(
                        vt[st][:, :, DEPTH:2 * DEPTH],
                        vpad[:],
                    )
                    nc.vector.tensor_copy(
                        vt[st][:, :, 0:DEPTH],
                        ps[:].rearrange("p (h e) -> p h e", h=HPC),
                    )

            # ---- Phase 2: attention, fully transposed ----
            # Inner loop: PE (logits, AV^T), ScalarE (exp psum->sbuf, the
            # pipeline wall at ~1.04us/tile), VectorE + GpSimd (mask multiply),
            # with the softmax normalize fused into the psum eviction.
            attnT = [pp.tile([128, S], bf16, tag=f"attnT{g}", name=f"attnT{g}") for g in range(2)]
            with tc.tile_pool(name="attn", bufs=2) as ab, \
                 tc.tile_pool(name="exs", bufs=3) as exs, \
                 tc.tile_pool(name="psL", bufs=2, space="PSUM") as psL, \
                 tc.tile_pool(name="psO", bufs=2, space="PSUM") as psO:
                for qcp in range(2):
                    qs = slice(qcp * 1024, (qcp + 1) * 1024)
                    for h in range(HPC):
                        g, po = h // 2, (h % 2) * 64
                        pso = psO.tile([128, 1024], f32, tag="av", name="av")
                        for kb in range(16):
                            psl = psL.tile([128, 1024], f32, tag="lg", name="lg")
                            for half in range(2):
                                hs = slice(half * 512, (half + 1) * 512)
                                qh = slice(qcp * 1024 + half * 512,
                                           qcp * 1024 + half * 512 + 512)
                                nc.tensor.matmul(
                                    psl[:, hs],
                                    kT[g][po:po + 64, kb * 128:(kb + 1) * 128],
                                    qT[g][po:po + 64, qh],
                                    start=True, stop=True,
                                )
                            ex = exs.tile([128, 1024], bf16, tag="ex", name="ex", bufs=4)
                            nc.scalar.activation(ex[:], psl[:], EXP, scale=0.125)
                            em = exs.tile([128, 1024], bf16, tag="em", name="em", bufs=8)
                            if kb in GPS_KB:
                                nc.gpsimd.tensor_tensor(em[:], ex[:], mt[:, kb, qs], MULT)
                            else:
                                nc.vector.tensor_tensor(em[:], ex[:], mt[:, kb, qs], MULT)
                            for half in range(2):
                                hs = slice(half * 512, (half + 1) * 512)
                                nc.tensor.matmul(
                                    pso[:, hs], vt[kb][:, h, :], em[:, hs],
                                    start=(kb == 0), stop=(kb == 15),
                                )
                        # Softmax denominators sit on psum row 64: reciprocal
                        # straight off psum, broadcast across the 64 head dims
                        # on GpSimd, and normalize during the eviction.
                        rden = ab.tile([1, 1024], f32, tag="rden", name="rden")
                        nc.vector.reciprocal(rden[:], pso[64:65, :])
                        rb = ab.tile([64, 1024], f32, tag="rb", name="rb")
                        nc.gpsimd.partition_broadcast(rb[:], rden[:])
                        nc.vector.tensor_tensor(
                            attnT[g][po:po + 64, qs], pso[0:64, :], rb[:], MULT,
                        )

                    # Output projection for this qcp's s-range (overlaps
                    # with the next qcp's attention on the other engines).
                    for st in range(qcp * 8, qcp * 8 + 8):
                        psf = psO.tile([128, 1024], f32, tag="av", name="po")
                        for nch in range(2):
                            ns = slice(nch * 512, (nch + 1) * 512)
                            for g in range(2):
                                nc.tensor.matmul(
                                    psf[:, ns],
                                    attnT[g][:, st * 128:(st + 1) * 128],
                                    wot[g][:, ns],
                                    start=(g == 0), stop=(g == 1),
                                )
                        ot = ab.tile([128, D], bf16, tag="ot", name="ot")
                        nc.vector.tensor_copy(ot[:], psf[:])
                        nc.sync.dma_start(out[st * 128:(st + 1) * 128, :], ot[:])

    nc.compile()
    nc.m = get_hw_module(nc.m)
    return nc


def _get_program():
    if "nc" not in _CACHE:
        _CACHE["nc"] = _build_program()
    return _CACHE["nc"]


def _make_in_maps(query, attention_mask, Wq, Wk, Wv, Wo):
    import ml_dtypes

    bf16 = ml_dtypes.bfloat16
    in_maps = []
    imaskT_b = []
    xT_b = []
    for b in range(B):
        imaskT_b.append(
            np.ascontiguousarray(1 - attention_mask[b, 0].T).astype(bf16)
        )
        xT_b.append(np.ascontiguousarray(query[b].T).astype(bf16))
    vpad_np = np.zeros((128, HPC, DEPTH), dtype=bf16)
    vpad_np[:, :, 0] = 1.0
    for c in range(CORES):
        b, hg = c // HG, c % HG
        cs = slice(hg * DH, (hg + 1) * DH)
        in_maps.append({
            "xT": xT_b[b],
            "imaskT": imaskT_b[b],
            "wq": np.ascontiguousarray(Wq[:, cs]).astype(bf16),
            "wk": np.ascontiguousarray(Wk[:, cs]).astype(bf16),
            "wv": np.ascontiguousarray(Wv[:, cs]).astype(bf16),
            "wo": np.ascontiguousarray(Wo[cs, :]).astype(bf16),
            "vpad": vpad_np,
        })
    return in_maps


def _run(inputs, trace=False):
    from concourse.bass_utils import run_bass_kernel_spmd

    nc = _get_program()
    in_maps = _make_in_maps(**inputs)
    res = run_bass_kernel_spmd(
        nc, in_maps, core_ids=list(range(CORES)), trace=trace,
    )
    outs = [res.results[c]["out"].astype(np.float64) for c in range(CORES)]
    full = np.empty((B, S, D), dtype=np.float32)
    for b in range(B):
        acc = outs[4 * b]
        for hg in range(1, HG):
            acc = acc + outs[4 * b + hg]
        full[b] = acc.astype(np.float32)
    return full, res


def kernel(query, attention_mask, Wq, Wk, Wv, Wo):
    full, _ = _run(dict(
        query=np.asarray(query), attention_mask=np.asarray(attention_mask),
        Wq=np.asarray(Wq), Wk=np.asarray(Wk), Wv=np.asarray(Wv),
        Wo=np.asarray(Wo),
    ))
    return full


# revision 8
# speedup vs baseline: 1.1935x; 1.1935x over previous
"""Multi-head attention forward on 8 Trainium2 NeuronCores (Bass/Tile).

Problem: B=2, S=2048, d_model=1024, 16 heads (depth 64), fp32.
  q/k/v = query @ W{q,k,v}; logits = q k^T / 8 + mask * -1e9;
  out = softmax(logits) v @ Wo.

Sharding (Megatron-style, hardcoded): core c handles batch b = c//4 and head
group hg = c%4 (4 heads = 256 of the 1024 head dims). Wq/Wk/Wv are
column-sharded, Wo row-sharded; each core emits a partial [S, 1024] output and
the host sums the 4 partials per batch (the "all-reduce").

Per-core kernel design (all matmul operands bf16, f32 PSUM accumulate):
  * All attention math runs transposed: qT/kT are [depth, S] so QK^T lands as
    logitsT [k, q] tiles straight off the PE, and AV^T = V.T(lhsT) @ expT
    needs no transposes either.
  * The inner loop is paced by ScalarE exp (~1.04us per [128,1024] tile).
    The PE must stay ~100% duty or HAM throttles it to half clock, so all
    remaining PE work (g1 q/k projections, the output projection of the
    previous q-chunk, plus tiny identity spins where nothing real is
    available) is interleaved into the attention iterations as filler.
  * The softmax denominator comes free from a ones-column appended to V
    (row 64 of the AV psum accumulator). Its reciprocal runs partition-major:
    a partition-scatter DMA reshapes the psum row to [128,8], VectorE
    reciprocals it, a partition-gather DMA flattens it back, GpSimd
    broadcasts it across 64 partitions, and a deferred in-place multiply
    normalizes attnT a few iterations later.
  * The output projection DMAs straight from PSUM to HBM (no eviction copy).
  * The mask streams in 16 split DMAs gated behind the last weight DMA (a
    1-element copy creates the dependency) so it cannot steal bandwidth from
    the projections' inputs; a short identity-matmul spin warms the PE clock
    while the first DMAs land.
"""

import sys

import numpy as np

sys.path.insert(0, "/opt/trn_rl_repo")

B = 2
S = 2048
D = 1024
HEADS = 16
DEPTH = 64
CORES = 8
HG = 4          # head groups (cores per batch)
HPC = 4         # heads per core
DH = HPC * DEPTH  # per-core head width = 256

_CACHE = {}


def _build_program():
    import concourse.bass as bass  # noqa: F401  (registers engines)
    import concourse.mybir as mybir
    import concourse.tile as tile
    from concourse import bacc, library_config
    from concourse.bass_interp import get_hw_module
    from concourse.masks import make_identity

    dt = mybir.dt
    f32, bf16 = dt.float32, dt.bfloat16
    MULT = mybir.AluOpType.mult
    EXP = mybir.ActivationFunctionType.Exp

    nc = bacc.Bacc(
        "TRN2",
        target_bir_lowering=False,
        debug=False,
        enable_asserts=True,
        num_devices=CORES,
    )

    xT = nc.dram_tensor("xT", [D, S], bf16, kind="ExternalInput").ap()
    imaskT = nc.dram_tensor("imaskT", [S, S], bf16, kind="ExternalInput").ap()
    wq = nc.dram_tensor("wq", [D, DH], bf16, kind="ExternalInput").ap()
    wk = nc.dram_tensor("wk", [D, DH], bf16, kind="ExternalInput").ap()
    wv = nc.dram_tensor("wv", [D, DH], bf16, kind="ExternalInput").ap()
    wo = nc.dram_tensor("wo", [DH, D], bf16, kind="ExternalInput").ap()
    vones = nc.dram_tensor("vones", [128, HPC, 1], bf16, kind="ExternalInput").ap()
    out = nc.dram_tensor("out", [S, D], bf16, kind="ExternalOutput").ap()

    with tile.TileContext(nc) as tc:
        nc.gpsimd.load_library(library_config.proxy)
        with tc.tile_pool(name="persist", bufs=1) as pp:
            qT = [pp.tile([128, S], bf16, tag=f"qT{g}", name=f"qT{g}") for g in range(2)]
            kT = [pp.tile([128, S], bf16, tag=f"kT{g}", name=f"kT{g}") for g in range(2)]
            vt = [pp.tile([128, HPC, DEPTH + 1], bf16, tag=f"v{i}", name=f"v{i}") for i in range(16)]
            wot = [pp.tile([128, D], bf16, tag=f"wo{g}", name=f"wo{g}") for g in range(2)]
            attnT = [pp.tile([128, S], bf16, tag=f"attnT{g}", name=f"attnT{g}") for g in range(2)]
            mt = pp.tile([128, 16, S], bf16, tag="mask", name="mask")
            ident = pp.tile([128, 128], f32, tag="ident", name="ident")

            make_identity(nc, ident[:])
            with tc.tile_pool(name="psW", bufs=2, space="PSUM") as psW:
                for w in range(80):
                    psw = psW.tile([128, 128], f32, tag="warm", name="warm")
                    nc.tensor.matmul(psw[:], ident[:], ident[:],
                                     start=True, stop=True)

            with tc.tile_pool(name="xw", bufs=1) as xw, \
                 tc.tile_pool(name="attn", bufs=2) as ab, \
                 tc.tile_pool(name="exs", bufs=3) as exs, \
                 tc.tile_pool(name="psL", bufs=2, space="PSUM") as psL, \
                 tc.tile_pool(name="psO", bufs=1, space="PSUM") as psO, \
                 tc.tile_pool(name="psP", bufs=2, space="PSUM") as psP:

                # ---- Input DMAs. xt is split in halves for better queue
                # spread; the mask is gated behind wv's last chunk.
                xt = [xw.tile([128, S], bf16, tag=f"x{d}", name=f"x{d}") for d in range(8)]
                wts = {}
                for nm in ("wq", "wk", "wv"):
                    wts[nm] = [xw.tile([128, DH], bf16, tag=f"{nm}{d}", name=f"{nm}{d}") for d in range(8)]
                for d in range(8):
                    nc.sync.dma_start(wts["wq"][d][:], wq[d * 128:(d + 1) * 128, :])
                for d in range(8):
                    nc.sync.dma_start(xt[d][:, 0:1024], xT[d * 128:(d + 1) * 128, 0:1024])
                    nc.sync.dma_start(xt[d][:, 1024:2048], xT[d * 128:(d + 1) * 128, 1024:2048])
                for nm, srcd in (("wk", wk), ("wv", wv)):
                    for d in range(8):
                        nc.sync.dma_start(wts[nm][d][:], srcd[d * 128:(d + 1) * 128, :])
                for g in range(2):
                    nc.sync.dma_start(wot[g][:], wo[g * 128:(g + 1) * 128, :])
                imaskT_r = imaskT.rearrange("(t p) q -> p t q", p=128)
                for kb in range(16):
                    # 1-element copy: makes the mask DMA wait for the last
                    # weight chunk so it can't steal input bandwidth.
                    nc.vector.tensor_copy(mt[0:1, kb, 0:1], wts["wv"][7][0:1, 0:1])
                    nc.sync.dma_start(mt[:, kb:kb + 1, :], imaskT_r[:, kb:kb + 1, :])

                # ---- PE work units (also used as attention-loop filler) ----
                def emit_qk_group(nm, dst, g, sc):
                    ps = psP.tile([128, 512], f32, tag="pj", name="pg")
                    for d in range(8):
                        nc.tensor.matmul(
                            ps[:],
                            wts[nm][d][:, g * 128:(g + 1) * 128],
                            xt[d][:, sc * 512:(sc + 1) * 512],
                            start=(d == 0), stop=(d == 7),
                        )
                    nc.vector.tensor_copy(dst[g][:, sc * 512:(sc + 1) * 512], ps[:])

                def emit_v_group(st):
                    ps = psP.tile([128, DH], f32, tag="pj", name="pv")
                    for d in range(8):
                        nc.tensor.matmul(
                            ps[:],
                            xt[d][:, st * 128:(st + 1) * 128],
                            wts["wv"][d][:],
                            start=(d == 0), stop=(d == 7),
                        )
                    nc.sync.dma_start(vt[st][:, :, DEPTH:DEPTH + 1], vones[:])
                    nc.vector.tensor_copy(
                        vt[st][:, :, 0:DEPTH],
                        ps[:].rearrange("p (h e) -> p h e", h=HPC),
                    )

                def emit_outproj_unit(st, nch):
                    ns = slice(nch * 512, (nch + 1) * 512)
                    psf = psP.tile([128, 512], f32, tag="pj", name="po")
                    for g in range(2):
                        nc.tensor.matmul(
                            psf[:],
                            attnT[g][:, st * 128:(st + 1) * 128],
                            wot[g][:, ns],
                            start=(g == 0), stop=(g == 1),
                        )
                    ot = ab.tile([128, 512], bf16, tag="ot", name="ot")
                    nc.vector.tensor_copy(ot[:], psf[:])
                    nc.sync.dma_start(out[st * 128:(st + 1) * 128, ns], ot[:])

                def emit_dummy():
                    psd = psP.tile([128, 128], f32, tag="pj", name="dum")
                    nc.tensor.matmul(psd[:], ident[:], ident[:],
                                     start=True, stop=True)

                # ---- Prologue: g0 q/k projections + all of V.
                for sc in range(4):
                    emit_qk_group("wk", kT, 0, sc)
                    emit_qk_group("wq", qT, 0, sc)
                for st in range(16):
                    emit_v_group(st)

                # g1 projections feed h2/h3; k chunks first (consumed in kb
                # order), q's qcp0 halves next, qcp1 halves last.
                g1_queue = [("wk", kT, 1, 0), ("wq", qT, 1, 0), ("wq", qT, 1, 1),
                            ("wk", kT, 1, 1), ("wk", kT, 1, 2), ("wk", kT, 1, 3),
                            ("wq", qT, 1, 2), ("wq", qT, 1, 3)]

                def emit_head_epilogue(qcp, h, pso):
                    g, po = h // 2, (h % 2) * 64
                    qs = slice(qcp * 1024, (qcp + 1) * 1024)
                    dsb = ab.tile([1, 1024], f32, tag="dsb", name="dsb")
                    nc.vector.tensor_copy(dsb[:], pso[64:65, :])
                    nc.vector.tensor_copy(attnT[g][po:po + 64, qs], pso[0:64, :])
                    dT = ab.tile([128, 8], f32, tag="dT", name="dT")
                    nc.sync.dma_start(dT[:, :], dsb[:, :])
                    rT = ab.tile([128, 8], f32, tag="rT", name="rT")
                    nc.vector.reciprocal(rT[:], dT[:])
                    rden = ab.tile([1, 1024], f32, tag="rden", name="rden")
                    nc.sync.dma_start(rden[:, :], rT[:, :])
                    rb = ab.tile([128, 1024], f32, tag="rb", name="rb")
                    nc.gpsimd.partition_broadcast(rb[:], rden[:])
                    return (g, po, qs, rb)

                def emit_norm(pend):
                    g, po, qs, rb = pend
                    nc.gpsimd.tensor_tensor(
                        attnT[g][po:po + 64, qs],
                        attnT[g][po:po + 64, qs], rb[po:po + 64, :], MULT,
                    )

                # ---- Attention superloop ----
                pending_norm = None
                op_queue = []
                for qcp in range(2):
                    qs = slice(qcp * 1024, (qcp + 1) * 1024)
                    for h in range(HPC):
                        g, po = h // 2, (h % 2) * 64
                        pso = psO.tile([65, 1024], f32, tag="av", name="av")
                        for kb in range(16):
                            psl = psL.tile([128, 1024], f32, tag="lg", name="lg")
                            for half in range(2):
                                hs = slice(half * 512, (half + 1) * 512)
                                qh = slice(qcp * 1024 + half * 512,
                                           qcp * 1024 + half * 512 + 512)
                                nc.tensor.matmul(
                                    psl[:, hs],
                                    kT[g][po:po + 64, kb * 128:(kb + 1) * 128],
                                    qT[g][po:po + 64, qh],
                                    start=True, stop=True,
                                )
                            ex = exs.tile([128, 1024], bf16, tag="ex", name="ex", bufs=4)
                            nc.scalar.activation(ex[:], psl[:], EXP, scale=0.125)
                            em = exs.tile([128, 1024], bf16, tag="em", name="em", bufs=8)
                            if kb in (2, 7, 12):
                                nc.gpsimd.tensor_tensor(em[:], ex[:], mt[:, kb, qs], MULT)
                            else:
                                nc.vector.tensor_tensor(em[:], ex[:], mt[:, kb, qs], MULT)
                            for half in range(2):
                                hs = slice(half * 512, (half + 1) * 512)
                                nc.tensor.matmul(
                                    pso[:, hs], vt[kb][:, h, :], em[:, hs],
                                    start=(kb == 0), stop=(kb == 15),
                                )
                            # Interleaved PE filler + deferred normalize.
                            if kb == 4 and pending_norm is not None:
                                emit_norm(pending_norm)
                                pending_norm = None
                            if qcp == 0 and h < 2 and kb % 4 == 3 and g1_queue:
                                emit_qk_group(*g1_queue.pop(0))
                            if qcp == 0 and h >= 2:
                                emit_dummy()
                            if qcp == 1 and kb in (5, 8, 11, 14) and op_queue:
                                emit_outproj_unit(*op_queue.pop(0))
                        pending_norm = emit_head_epilogue(qcp, h, pso)
                    if qcp == 0:
                        op_queue = [(st, nch) for st in range(8) for nch in range(2)]

                # ---- Tail: last head's normalize + qcp1's output projection.
                emit_norm(pending_norm)
                for st, nch in op_queue:
                    emit_outproj_unit(st, nch)
                for st in range(8, 16):
                    for nch in range(2):
                        emit_outproj_unit(st, nch)

    nc.compile()
    nc.m = get_hw_module(nc.m)
    return nc


def _get_program():
    if "nc" not in _CACHE:
        _CACHE["nc"] = _build_program()
    return _CACHE["nc"]


def _make_in_maps(query, attention_mask, Wq, Wk, Wv, Wo):
    import ml_dtypes

    bf16 = ml_dtypes.bfloat16
    in_maps = []
    imaskT_b = []
    xT_b = []
    for b in range(B):
        imaskT_b.append(
            np.ascontiguousarray(1 - attention_mask[b, 0].T).astype(bf16)
        )
        xT_b.append(np.ascontiguousarray(query[b].T).astype(bf16))
    vones_np = np.ones((128, HPC, 1), dtype=bf16)
    for c in range(CORES):
        b, hg = c // HG, c % HG
        cs = slice(hg * DH, (hg + 1) * DH)
        in_maps.append({
            "xT": xT_b[b],
            "imaskT": imaskT_b[b],
            "wq": np.ascontiguousarray(Wq[:, cs]).astype(bf16),
            "wk": np.ascontiguousarray(Wk[:, cs]).astype(bf16),
            "wv": np.ascontiguousarray(Wv[:, cs]).astype(bf16),
            "wo": np.ascontiguousarray(Wo[cs, :]).astype(bf16),
            "vones": vones_np,
        })
    return in_maps


def _run(inputs, trace=False):
    from concourse.bass_utils import run_bass_kernel_spmd

    nc = _get_program()
    in_maps = _make_in_maps(**inputs)
    res = run_bass_kernel_spmd(
        nc, in_maps, core_ids=list(range(CORES)), trace=trace,
    )
    outs = [res.results[c]["out"].astype(np.float64) for c in range(CORES)]
    full = np.empty((B, S, D), dtype=np.float32)
    for b in range(B):
        acc = outs[4 * b]
        for hg in range(1, HG):
            acc = acc + outs[4 * b + hg]
        full[b] = acc.astype(np.float32)
    return full, res


def kernel(query, attention_mask, Wq, Wk, Wv, Wo):
    full, _ = _run(dict(
        query=np.asarray(query), attention_mask=np.asarray(attention_mask),
        Wq=np.asarray(Wq), Wk=np.asarray(Wk), Wv=np.asarray(Wv),
        Wo=np.asarray(Wo),
    ))
    return full


# revision 9
# speedup vs baseline: 1.5131x; 1.2678x over previous
"""Multi-head attention forward on 8 Trainium2 NeuronCores (Bass/Tile).

Problem: B=2, S=2048, d_model=1024, 16 heads (depth 64), fp32.
  q/k/v = query @ W{q,k,v}; logits = q k^T / 8 + mask * -1e9;
  out = softmax(logits) v @ Wo.

Sharding (Megatron-style, hardcoded): core c handles batch b = c//4 and head
group hg = c%4 (4 heads = 256 of the 1024 head dims). Wq/Wk/Wv are
column-sharded, Wo row-sharded; each core emits a partial [S, 1024] output and
the host sums the 4 partials per batch (the "all-reduce").

Per-core kernel design (all matmul operands bf16, f32 PSUM accumulate):
  * All attention math runs transposed: qT/kT are [depth, S] so QK^T lands as
    logitsT [k, q] tiles straight off the PE, and AV^T = V.T(lhsT) @ expT
    needs no transposes either.
  * Every weight operand is a full [128,128] tile so the PE's fast-weight-load
    path hides LDWEIGHTS: QK uses the full two-head kT block as weights with
    zero-padded per-head qT copies as the moving operand, and V tiles are
    padded to 128 columns (ones at 64, zeros above) for the AV matmul.
  * The inner loop is paced by ScalarE exp (~1.08us per [128,1024] tile).
    The PE must stay ~100% duty or HAM throttles it to half clock, so the
    remaining PE work (g1 q/k projections, the previous q-chunk's output
    projection, tiny AV-repeat spins) is interleaved into the attention
    iterations as filler.
  * The softmax denominator comes free from the ones-column (psum row 64).
    Its reciprocal runs partition-major: copy the row out, partition-scatter
    DMA to [128,8], VectorE reciprocal, partition-gather DMA back, GpSimd
    broadcast across partitions, and a deferred in-place multiply normalizes
    attnT a few iterations later (VectorE for the last head so the tail
    doesn't wait on the GpSimd queue).
  * The mask streams in 16 split DMAs gated behind the last weight DMA (a
    1-element copy creates the dependency) so it cannot steal bandwidth from
    the projections' inputs; identity-matmul spins warm the PE clock while
    the first DMAs land. The tail output projection runs two-phase with four
    psum slots and evictions alternating between ScalarE and VectorE.
"""

import sys

import numpy as np

sys.path.insert(0, "/opt/trn_rl_repo")

B = 2
S = 2048
D = 1024
HEADS = 16
DEPTH = 64
CORES = 8
HG = 4          # head groups (cores per batch)
HPC = 4         # heads per core
DH = HPC * DEPTH  # per-core head width = 256

_CACHE = {}


def _build_program():
    import concourse.bass as bass  # noqa: F401  (registers engines)
    import concourse.mybir as mybir
    import concourse.tile as tile
    from concourse import bacc, library_config
    from concourse.bass_interp import get_hw_module
    from concourse.masks import make_identity

    dt = mybir.dt
    f32, bf16 = dt.float32, dt.bfloat16
    MULT = mybir.AluOpType.mult
    EXP = mybir.ActivationFunctionType.Exp

    nc = bacc.Bacc(
        "TRN2",
        target_bir_lowering=False,
        debug=False,
        enable_asserts=True,
        num_devices=CORES,
    )

    xT = nc.dram_tensor("xT", [D, S], bf16, kind="ExternalInput").ap()
    imaskT = nc.dram_tensor("imaskT", [S, S], bf16, kind="ExternalInput").ap()
    wq = nc.dram_tensor("wq", [D, DH], bf16, kind="ExternalInput").ap()
    wk = nc.dram_tensor("wk", [D, DH], bf16, kind="ExternalInput").ap()
    wv = nc.dram_tensor("wv", [D, DH], bf16, kind="ExternalInput").ap()
    wo = nc.dram_tensor("wo", [DH, D], bf16, kind="ExternalInput").ap()
    vones = nc.dram_tensor("vones", [128, HPC, DEPTH], bf16, kind="ExternalInput").ap()
    out = nc.dram_tensor("out", [S, D], bf16, kind="ExternalOutput").ap()

    with tile.TileContext(nc) as tc:
        with tc.tile_pool(name="persist", bufs=1) as pp:
            # qz[h]: zero-padded per-head qT so QK weights are the full
            # [128,128] kT block (rows of the other head multiply zeros).
            qz = [pp.tile([128, S], bf16, tag=f"qz{h}", name=f"qz{h}") for h in range(HPC)]
            kT = [pp.tile([128, S], bf16, tag=f"kT{g}", name=f"kT{g}") for g in range(2)]
            vt = [pp.tile([128, HPC, 128], bf16, tag=f"v{i}", name=f"v{i}") for i in range(16)]
            wot = [pp.tile([128, D], bf16, tag=f"wo{g}", name=f"wo{g}") for g in range(2)]
            attnT = [pp.tile([128, S], bf16, tag=f"attnT{g}", name=f"attnT{g}") for g in range(2)]
            mt = pp.tile([128, 16, S], bf16, tag="mask", name="mask")
            ident = pp.tile([128, 128], f32, tag="ident", name="ident")

            make_identity(nc, ident[:])
            # Library load AFTER make_identity: the ucode reload stalls the
            # GpSimd queue ~11us and nothing needs it until the first head
            # epilogue.
            nc.gpsimd.load_library(library_config.attn)
            for h in range(HPC):
                po = (h % 2) * 64
                nc.vector.memset(qz[h][64 - po:128 - po, :], 0.0)

            with tc.tile_pool(name="psW", bufs=2, space="PSUM") as psW:
                for w in range(44):
                    psw = psW.tile([128, 128], f32, tag="warm", name="warm")
                    nc.tensor.matmul(psw[:], ident[:], ident[:],
                                     start=True, stop=True)

            with tc.tile_pool(name="xw", bufs=1) as xw, \
                 tc.tile_pool(name="attn", bufs=2) as ab, \
                 tc.tile_pool(name="exs", bufs=3) as exs, \
                 tc.tile_pool(name="psL", bufs=2, space="PSUM") as psL, \
                 tc.tile_pool(name="psO", bufs=1, space="PSUM") as psO, \
                 tc.tile_pool(name="psP", bufs=2, space="PSUM") as psP:

                # ---- Input DMAs. xt split in halves for queue spread; the
                # mask gated behind wv's last chunk.
                xt = [xw.tile([128, S], bf16, tag=f"x{d}", name=f"x{d}") for d in range(8)]
                wts = {}
                for nm in ("wq", "wk", "wv"):
                    wts[nm] = [xw.tile([128, DH], bf16, tag=f"{nm}{d}", name=f"{nm}{d}") for d in range(8)]
                for d in range(8):
                    nc.sync.dma_start(wts["wq"][d][:], wq[d * 128:(d + 1) * 128, :])
                for d in range(8):
                    nc.sync.dma_start(xt[d][:, 0:1024], xT[d * 128:(d + 1) * 128, 0:1024])
                    nc.sync.dma_start(xt[d][:, 1024:2048], xT[d * 128:(d + 1) * 128, 1024:2048])
                for nm, srcd in (("wk", wk), ("wv", wv)):
                    for d in range(8):
                        nc.sync.dma_start(wts[nm][d][:], srcd[d * 128:(d + 1) * 128, :])
                for g in range(2):
                    nc.sync.dma_start(wot[g][:], wo[g * 128:(g + 1) * 128, :])
                imaskT_r = imaskT.rearrange("(t p) q -> p t q", p=128)
                for kb in range(16):
                    nc.vector.tensor_copy(mt[0:1, kb, 0:1], wts["wv"][7][0:1, 0:1])
                    nc.sync.dma_start(mt[:, kb:kb + 1, :], imaskT_r[:, kb:kb + 1, :])

                # ---- PE work units (also used as attention-loop filler) ----
                def emit_qk_group(nm, g, sc):
                    ps = psP.tile([128, 512], f32, tag="pj", name="pg")
                    for d in range(8):
                        nc.tensor.matmul(
                            ps[:],
                            wts[nm][d][:, g * 128:(g + 1) * 128],
                            xt[d][:, sc * 512:(sc + 1) * 512],
                            start=(d == 0), stop=(d == 7),
                        )
                    cs = slice(sc * 512, (sc + 1) * 512)
                    if nm == "wq":
                        nc.vector.tensor_copy(qz[2 * g][0:64, cs], ps[0:64, :])
                        nc.vector.tensor_copy(qz[2 * g + 1][64:128, cs], ps[64:128, :])
                    else:
                        nc.vector.tensor_copy(kT[g][:, cs], ps[:])

                def emit_v_group(st):
                    ps = psP.tile([128, DH], f32, tag="pj", name="pv")
                    for d in range(8):
                        nc.tensor.matmul(
                            ps[:],
                            xt[d][:, st * 128:(st + 1) * 128],
                            wts["wv"][d][:],
                            start=(d == 0), stop=(d == 7),
                        )
                    nc.sync.dma_start(vt[st][:, :, DEPTH:128], vones[:])
                    nc.vector.tensor_copy(
                        vt[st][:, :, 0:DEPTH],
                        ps[:].rearrange("p (h e) -> p h e", h=HPC),
                    )

                def emit_outproj_unit(st, nch):
                    ns = slice(nch * 512, (nch + 1) * 512)
                    psf = psP.tile([128, 512], f32, tag="pj", name="po")
                    for g in range(2):
                        nc.tensor.matmul(
                            psf[:],
                            attnT[g][:, st * 128:(st + 1) * 128],
                            wot[g][:, ns],
                            start=(g == 0), stop=(g == 1),
                        )
                    ot = ab.tile([128, 512], bf16, tag="ot", name="ot", bufs=4)
                    nc.vector.tensor_copy(ot[:], psf[:])
                    nc.sync.dma_start(out[st * 128:(st + 1) * 128, ns], ot[:])

                # ---- Prologue: g0 q/k projections + all of V.
                for sc in range(4):
                    emit_qk_group("wk", 0, sc)
                    emit_qk_group("wq", 0, sc)
                for st in range(16):
                    emit_v_group(st)

                g1_queue = [("wk", 1, 0), ("wq", 1, 0), ("wq", 1, 1),
                            ("wk", 1, 1), ("wk", 1, 2), ("wk", 1, 3),
                            ("wq", 1, 2), ("wq", 1, 3)]

                def emit_head_epilogue(qcp, h, pso):
                    g, po = h // 2, (h % 2) * 64
                    qs = slice(qcp * 1024, (qcp + 1) * 1024)
                    dsb = ab.tile([1, 1024], bf16, tag="dsb", name="dsb")
                    nc.vector.tensor_copy(dsb[:], pso[64:65, :])
                    nc.vector.tensor_copy(attnT[g][po:po + 64, qs], pso[0:64, :])
                    dT = ab.tile([128, 8], bf16, tag="dT", name="dT")
                    nc.sync.dma_start(dT[:, :], dsb[:, :])
                    rT = ab.tile([128, 8], f32, tag="rT", name="rT")
                    nc.vector.reciprocal(rT[:], dT[:])
                    rden = ab.tile([1, 1024], f32, tag="rden", name="rden")
                    nc.sync.dma_start(rden[:, :], rT[:, :])
                    rb = ab.tile([128, 1024], f32, tag="rb", name="rb")
                    nc.gpsimd.partition_broadcast(rb[:], rden[:])
                    return (g, po, qs, rb)

                def emit_norm(pend, engine):
                    g, po, qs, rb = pend
                    engine.tensor_tensor(
                        attnT[g][po:po + 64, qs],
                        attnT[g][po:po + 64, qs], rb[po:po + 64, :], MULT,
                    )

                # ---- Attention superloop ----
                pending_norm = None
                op_queue = []
                for qcp in range(2):
                    qs = slice(qcp * 1024, (qcp + 1) * 1024)
                    for h in range(HPC):
                        g, po = h // 2, (h % 2) * 64
                        pso = psO.tile([128, 1024], f32, tag="av", name="av")
                        for kb in range(16):
                            psl = psL.tile([128, 1024], f32, tag="lg", name="lg")
                            for half in range(2):
                                hs = slice(half * 512, (half + 1) * 512)
                                qh = slice(qcp * 1024 + half * 512,
                                           qcp * 1024 + half * 512 + 512)
                                nc.tensor.matmul(
                                    psl[:, hs],
                                    kT[g][:, kb * 128:(kb + 1) * 128],
                                    qz[h][:, qh],
                                    start=True, stop=True,
                                )
                            ex = exs.tile([128, 1024], bf16, tag="ex", name="ex", bufs=3)
                            nc.scalar.activation(ex[:], psl[:], EXP, scale=0.125)
                            em = exs.tile([128, 1024], bf16, tag="em", name="em", bufs=6)
                            nc.vector.tensor_tensor(em[:], ex[:], mt[:, kb, qs], MULT)
                            for half in range(2):
                                hs = slice(half * 512, (half + 1) * 512)
                                nc.tensor.matmul(
                                    pso[:, hs], vt[kb][:, h, :], em[:, hs],
                                    start=(kb == 0), stop=(kb == 15),
                                )
                            # Interleaved PE filler + deferred normalize.
                            if kb == 4 and pending_norm is not None:
                                emit_norm(pending_norm, nc.gpsimd)
                                pending_norm = None
                            if qcp == 0 and h < 2 and kb % 4 == 3 and g1_queue:
                                emit_qk_group(*g1_queue.pop(0))
                            if qcp == 0 and h >= 2:
                                # AV-repeat spin: reuses loaded weights, keeps
                                # PE duty at 100% so HAM stays at full clock.
                                psd = psP.tile([128, 256], f32, tag="pj", name="dum")
                                nc.tensor.matmul(psd[:], vt[kb][:, h, :],
                                                 em[:, 0:256], start=True, stop=True)
                            if qcp == 1 and kb in (5, 8, 11, 14) and op_queue:
                                emit_outproj_unit(*op_queue.pop(0))
                        pending_norm = emit_head_epilogue(qcp, h, pso)
                    if qcp == 0:
                        op_queue = [(st, nch) for st in range(8) for nch in range(2)]

                # ---- Tail: last head's normalize (VectorE — the GpSimd
                # queue would add latency) + two-phase output projection.
                emit_norm(pending_norm, nc.vector)
                units = list(op_queue) + [(st, nch) for st in range(8, 16) for nch in range(2)]
                psfs = []
                for i, (st, nch) in enumerate(units):
                    ns = slice(nch * 512, (nch + 1) * 512)
                    pool, tag = (psP, "pj") if i % 2 == 0 else (psL, "lg")
                    psf = pool.tile([128, 512], f32, tag=tag, name="po")
                    nc.tensor.matmul(psf[:], attnT[0][:, st * 128:(st + 1) * 128],
                                     wot[0][:, ns], start=True, stop=False)
                    psfs.append((psf, st, ns, i))
                    keep = 2 if i < len(units) - 1 else 0
                    while len(psfs) > keep:
                        psf2, st2, ns2, j = psfs.pop(0)
                        nc.tensor.matmul(psf2[:], attnT[1][:, st2 * 128:(st2 + 1) * 128],
                                         wot[1][:, ns2], start=False, stop=True)
                        ot = ab.tile([128, 512], bf16, tag="ot", name="ot", bufs=4)
                        if j % 2 == 0:
                            nc.vector.tensor_copy(ot[:], psf2[:])
                        else:
                            nc.scalar.copy(ot[:], psf2[:])
                        nc.sync.dma_start(out[st2 * 128:(st2 + 1) * 128, ns2], ot[:])

    nc.compile()
    nc.m = get_hw_module(nc.m)
    return nc


def _get_program():
    if "nc" not in _CACHE:
        _CACHE["nc"] = _build_program()
    return _CACHE["nc"]


def _make_in_maps(query, attention_mask, Wq, Wk, Wv, Wo):
    import ml_dtypes

    bf16 = ml_dtypes.bfloat16
    in_maps = []
    imaskT_b = []
    xT_b = []
    for b in range(B):
        imaskT_b.append(
            np.ascontiguousarray(1 - attention_mask[b, 0].T).astype(bf16)
        )
        xT_b.append(np.ascontiguousarray(query[b].T).astype(bf16))
    vones_np = np.zeros((128, HPC, DEPTH), dtype=bf16)
    vones_np[:, :, 0] = 1.0
    for c in range(CORES):
        b, hg = c // HG, c % HG
        cs = slice(hg * DH, (hg + 1) * DH)
        in_maps.append({
            "xT": xT_b[b],
            "imaskT": imaskT_b[b],
            "wq": np.ascontiguousarray(Wq[:, cs]).astype(bf16),
            "wk": np.ascontiguousarray(Wk[:, cs]).astype(bf16),
            "wv": np.ascontiguousarray(Wv[:, cs]).astype(bf16),
            "wo": np.ascontiguousarray(Wo[cs, :]).astype(bf16),
            "vones": vones_np,
        })
    return in_maps


def _run(inputs, trace=False):
    from concourse.bass_utils import run_bass_kernel_spmd

    nc = _get_program()
    in_maps = _make_in_maps(**inputs)
    res = run_bass_kernel_spmd(
        nc, in_maps, core_ids=list(range(CORES)), trace=trace,
    )
    outs = [res.results[c]["out"].astype(np.float64) for c in range(CORES)]
    full = np.empty((B, S, D), dtype=np.float32)
    for b in range(B):
        acc = outs[4 * b]
        for hg in range(1, HG):
            acc = acc + outs[4 * b + hg]
        full[b] = acc.astype(np.float32)
    return full, res


def kernel(query, attention_mask, Wq, Wk, Wv, Wo):
    full, _ = _run(dict(
        query=np.asarray(query), attention_mask=np.asarray(attention_mask),
        Wq=np.asarray(Wq), Wk=np.asarray(Wk), Wv=np.asarray(Wv),
        Wo=np.asarray(Wo),
    ))
    return full
